# revision 26
# baseline (speedup 1.0000x reference)
"""Trainium2 Bass kernel: DynamicMoERoutingLayer (moe_routing).

Reference computes: routing projection -> cosine-sim vs 10 expert embeddings ->
softmax weights -> 10 expert 3x3 VALID convs -> weighted combine.

Key algebraic rewrite: conv is linear in its weights, so
    sum_n w[b,n] * conv(x_b, W_n)  ==  conv(x_b, sum_n w[b,n] * W_n)
We therefore combine the 10 expert kernels into ONE per-image kernel on device
(10x less conv compute), then run a single 3x3 conv per image.

Distribution: data-parallel over batch, 4 images per core (8 cores).
Each core processes its 4 images as 2 "pairs": images A/B of a pair live on
SBUF partitions 0-63 / 64-127 of the same tiles, and their matmuls run
concurrently on disjoint PE-array quadrants (row+col tiling), so K=64 matmuls
still use the full 128x128 array.

Conv-as-matmul: x is stored flat [cin, y*64+x] (4096 px incl. 2 garbage
columns/rows which the host trims). For each of the 9 taps (dy,dx) one
matmul per 512-px chunk accumulates W_tap^T @ x_shifted into PSUM.

Precision: x is cast fp32->bf16 during the DMA load (free, SWDGE cast);
combined weights are accumulated in fp32 then cast to bf16; conv matmuls are
bf16 with fp32 PSUM accumulation; routing math is all fp32.

Codegen constraint honored throughout: a Matmult can carry only ONE semaphore
wait, so every matmul's inputs must be one producer "behind" it. All small
constants ship in a single blob DMA; cheap touch-matmuls absorb the x-DMA /
weight-DVE ticks before the real matmul streams begin.
"""

import functools
import os
import sys

import numpy as np

for _p in ("/opt/trn_rl_repo",):
    if os.path.isdir(_p) and _p not in sys.path:
        sys.path.insert(0, _p)

import concourse.bacc as bacc
import concourse.bass as bass
import concourse.mybir as mybir
import concourse.tile as tile
from concourse.bass_utils import run_bass_kernel_spmd

FP = mybir.dt.float32
FR = mybir.dt.float32r
BF = mybir.dt.bfloat16
AF = mybir.ActivationFunctionType
OP = mybir.AluOpType

N_CORES = 8
B = 32
B_LOC = B // N_CORES          # images per core
NPAIR = B_LOC // 2
CIN = 64
COUT = 64
PIX = 64 * 64                 # flat pixels computed per image (incl. garbage)
XCOLS = 4352                  # padded x columns (>= 4095 + 130 + 1)
XPAD = 256                    # tail pad elements of the flat x upload
NEXP = 10
D = 128
R = 512
CHUNK = 512
NCHUNK = PIX // CHUNK         # 8
TAPS = 9
WAVE = 3                      # chunks per wave (PSUM banks: 6 conv + 2 routing)

# const-blob column layout
C_RPW = 0                     # [128, 4, 128]
C_RV = 512                    # [128, 4, 4]
C_RPB = 528                   # [128, 1]
C_ID = 529                    # [128, 128] identity
C_EMB = 657                   # [10, 128] on partitions 0..9
C_SEL = 785                   # [4, 2, 128] on partitions 0..3
C_CBT = 1041                  # [128, 10]
CBLOB = 1056


def build_nc():
    # Bacc (not raw Bass): its compile() runs move_matmul_waits_to_ldweights +
    # generate_event_semaphores, which legalize multi-wait instructions for
    # the walrus ISA (each instruction carries at most one sync wait).
    nc = bacc.Bacc(None)

    x_d = nc.dram_tensor("x", [B_LOC * CIN * PIX + XPAD], FP,
                         kind="ExternalInput")
    cst_d = nc.dram_tensor("cst", [128, CBLOB], FP, kind="ExternalInput")
    base_d = nc.dram_tensor("base", [CIN, NEXP, TAPS * COUT], FP,
                            kind="ExternalInput")
    out_d = nc.dram_tensor("out", [B_LOC, COUT, PIX], FP, kind="ExternalOutput")

    with tile.TileContext(nc) as tc:
        with (
            tc.tile_pool(name="consts", bufs=1) as consts,
            tc.tile_pool(name="xp", bufs=2) as xp,
            tc.tile_pool(name="cwp", bufs=2) as cwp,
            tc.tile_pool(name="outp", bufs=2) as outp,
            tc.tile_pool(name="scr", bufs=1) as scr,
            tc.tile_pool(name="rps", bufs=2, space="PSUM") as rps,
            tc.tile_pool(name="cps", bufs=2 * WAVE, space="PSUM") as cps,
        ):
            # ---- constant loads (one DMA each => one wait for consumers) --
            cst = consts.tile([128, CBLOB], FP)
            nc.sync.dma_start(out=cst, in_=cst_d[:])
            rpw_t = cst[:, C_RPW:C_RPW + 512].rearrange("p (k d) -> p k d", k=4)
            rv_t = cst[:, C_RV:C_RV + 16].rearrange("p (k b) -> p k b", k=4)
            rpb_t = cst[:, C_RPB:C_RPB + 1]
            ident = cst[:, C_ID:C_ID + 128]
            emb_t = cst[0:NEXP, C_EMB:C_EMB + 128]
            sel_t = cst[0:B_LOC, C_SEL:C_SEL + 256].rearrange(
                "b (p q) -> b p q", p=NPAIR)
            cbt_t = cst[:, C_CBT:C_CBT + NEXP]

            # expert weight base, duplicated onto both partition halves by
            # reading the DRAM region twice (leading 0-stride dim).
            # HWDGE (sync) — SWDGE descriptor generation on the Q7 costs
            # ~7 us per dma_start for these shapes.
            base_t = consts.tile([128, NEXP, TAPS * COUT], FP)
            bsrc = base_d[:]
            bdup = bass.AP(tensor=bsrc.tensor, offset=bsrc.offset,
                           ap=[[0, 2]] + list(bsrc.ap))
            nc.sync.dma_start(out=base_t, in_=bdup)

            # ---- routing: r = rv @ rp_w.T + rp_b  (D on partitions) -------
            r_ps = rps.tile([128, B_LOC], FP, tag="r")
            for k0 in range(R // 128):
                nc.tensor.matmul(r_ps, lhsT=rpw_t[:, k0, :], rhs=rv_t[:, k0, :],
                                 start=(k0 == 0), stop=(k0 == R // 128 - 1))
            rT = scr.tile([128, B_LOC], FP)
            nc.vector.tensor_scalar(out=rT, in0=r_ps, scalar1=rpb_t,
                                    scalar2=None, op0=OP.add)

            # ||r_b||: transpose r to [b, d] then square+row-sum
            r4_ps = rps.tile([B_LOC, 128], FP, tag="r")
            nc.tensor.transpose(r4_ps, rT, ident)
            r4 = scr.tile([B_LOC, 128], FP)
            nc.vector.tensor_copy(r4, r4_ps)
            rsq = scr.tile([B_LOC, 128], FP)
            rn2 = scr.tile([B_LOC, 1], FP)
            nc.vector.scalar_tensor_tensor(out=rsq, in0=r4, scalar=1.0,
                                           in1=r4, op0=OP.mult, op1=OP.mult,
                                           accum_out=rn2)
            rnorm = scr.tile([B_LOC, 1], FP)
            nc.scalar.activation(out=rnorm, in_=rn2, func=AF.Sqrt)
            rinv = scr.tile([B_LOC, 1], FP)
            nc.vector.reciprocal(rinv, rnorm)

            # normalized embeddings, then transpose to [d, n]
            esq = scr.tile([NEXP, D], FP)
            en2 = scr.tile([NEXP, 1], FP)
            nc.vector.scalar_tensor_tensor(out=esq, in0=emb_t, scalar=1.0,
                                           in1=emb_t, op0=OP.mult, op1=OP.mult,
                                           accum_out=en2)
            enorm = scr.tile([NEXP, 1], FP)
            nc.scalar.activation(out=enorm, in_=en2, func=AF.Sqrt)
            einv = scr.tile([NEXP, 1], FP)
            nc.vector.reciprocal(einv, enorm)
            ehat = scr.tile([NEXP, D], FP)
            nc.vector.tensor_scalar(out=ehat, in0=emb_t, scalar1=einv,
                                    scalar2=None, op0=OP.mult)
            ehatT_ps = rps.tile([D, NEXP], FP, tag="r")
            nc.tensor.transpose(ehatT_ps, ehat, ident[:NEXP, :NEXP])
            ehatT = scr.tile([D, NEXP], FP)
            nc.vector.tensor_copy(ehatT, ehatT_ps)

            # cosine sim [b, n] and softmax over n
            dot_ps = rps.tile([B_LOC, NEXP], FP, tag="r")
            nc.tensor.matmul(dot_ps, lhsT=rT, rhs=ehatT, start=True, stop=True)
            sim = scr.tile([B_LOC, NEXP], FP)
            nc.vector.tensor_scalar(out=sim, in0=dot_ps, scalar1=rinv,
                                    scalar2=None, op0=OP.mult)
            nmax = scr.tile([B_LOC, 1], FP)
            nc.vector.tensor_reduce(out=nmax, in_=sim,
                                    axis=mybir.AxisListType.X, op=OP.max,
                                    negate=True)
            ex = scr.tile([B_LOC, NEXP], FP)
            sume = scr.tile([B_LOC, 1], FP)
            nc.scalar.activation(out=ex, in_=sim, func=AF.Exp,
                                 bias=nmax[:, 0:1], scale=1.0, accum_out=sume)
            sinv = scr.tile([B_LOC, 1], FP)
            nc.vector.reciprocal(sinv, sume)
            wts = scr.tile([B_LOC, NEXP], FP)
            nc.vector.tensor_scalar(out=wts, in0=ex, scalar1=sinv,
                                    scalar2=None, op0=OP.mult)

            # routing weights broadcast to all 128 partitions via selector
            # matmul: w128[part, p, n] = wts[2p + part//64, n]
            w128_ps = rps.tile([128, NPAIR, NEXP], FP, tag="r")
            for p in range(NPAIR):
                nc.tensor.matmul(w128_ps[:, p, :], lhsT=sel_t[:, p, :],
                                 rhs=wts, start=True, stop=True)
            w128 = consts.tile([128, NPAIR, NEXP], FP)
            nc.vector.tensor_copy(w128, w128_ps)

            # combined conv bias, pair-stacked [128, pair]:
            # bias2[part, p] = sum_n w128[part, p, n] * conv_b[n, part%64]
            bias2 = consts.tile([128, NPAIR], FP)
            bscrap = scr.tile([128, NEXP], FP)
            for p in range(NPAIR):
                nc.vector.scalar_tensor_tensor(
                    out=bscrap, in0=w128[:, p, :], scalar=1.0, in1=cbt_t,
                    op0=OP.mult, op1=OP.mult, accum_out=bias2[:, p:p + 1])

            # ---- per-pair conv ------------------------------------------
            xfull = x_d[:]
            for p in range(NPAIR):
                # plain HWDGE fp32 load; conv matmuls run as float32r
                # (full-rate fp32 at N>=256) so no engine cast is needed.
                # The BIR verifier wants every producer of fp32r-matmul
                # operands tagged float32r, so the tiles are FR and the DMA
                # source is bitcast (same bytes).
                xs = xp.tile([128, XCOLS], FP, name="xs")
                xsrc = bass.AP(tensor=xfull.tensor,
                               offset=xfull.offset + p * 128 * PIX,
                               ap=[[PIX, 128], [1, XCOLS]])
                # rows overlap-read so the pad columns carry real (finite)
                # neighboring data; their outputs are garbage and trimmed
                nc.sync.dma_start(out=xs, in_=xsrc)
                # fp32 -> bf16 on ScalarE (runs parallel to the DVE MACs);
                # bf16 matmuls stream 1 cyc/row and allow any tile_position
                # (fp32r is rejected at tile_position != (0,0))
                xtr = xp.tile([128, XCOLS], BF, name="xtr")
                nc.scalar.activation(out=xtr, in_=xs, func=AF.Copy)

                # combined per-image conv weights (fp32 MAC over experts)
                cw = cwp.tile([128, TAPS * COUT], FP)
                nc.vector.tensor_scalar(out=cw, in0=base_t[:, 0, :],
                                        scalar1=w128[:, p, 0:1], scalar2=None,
                                        op0=OP.mult)
                for n in range(1, NEXP):
                    nc.vector.scalar_tensor_tensor(
                        out=cw, in0=base_t[:, n, :], scalar=w128[:, p, n:n + 1],
                        in1=cw, op0=OP.mult, op1=OP.add)
                cwb = cwp.tile([128, TAPS * COUT], BF)
                nc.vector.tensor_copy(cwb, cw)

                outt = outp.tile([128, PIX], FP)
                for w0 in range(0, NCHUNK, WAVE):
                    chunks = list(range(w0, min(w0 + WAVE, NCHUNK)))
                    pst = {c: cps.tile([128, CHUNK], FP, name="pst")
                           for c in chunks}
                    # A Matmult can carry only ONE sync wait in the ISA, so
                    # absorb all cross-engine deps (psum bank release, x DMA,
                    # cwb DVE tick) into a PE-queue NOP first.  APs on sync
                    # instructions are the Tile-sanctioned dep mechanism
                    # (stripped at lowering); listing the psum tiles as OUTS
                    # makes the nop their allocating writer so the bank
                    # release waits land here, not on the first matmul.
                    dep = mybir.InstNoOp(
                        name=nc.get_next_instruction_name(),
                        text_hint="dep",
                        ins=[nc.tensor.lower_ap(xtr[:, 0:1]),
                             nc.tensor.lower_ap(cwb[:, 0:1])],
                        outs=[nc.tensor.lower_ap(pst[c]) for c in chunks],
                    )
                    nc.tensor.add_instruction(dep)
                    for t in range(TAPS):
                        off = (t // 3) * 64 + (t % 3)
                        for c in chunks:
                            lo = c * CHUNK + off
                            for half in (0, 1):
                                sl = slice(64 * half, 64 * half + 64)
                                nc.tensor.matmul(
                                    pst[c][sl, :],
                                    lhsT=cwb[sl, t * COUT:(t + 1) * COUT],
                                    rhs=xtr[sl, lo:lo + CHUNK],
                                    start=(t == 0), stop=(t == TAPS - 1),
                                    skip_group_check=True)
                    for c in chunks:
                        nc.scalar.activation(
                            out=outt[:, c * CHUNK:(c + 1) * CHUNK],
                            in_=pst[c], func=AF.Identity,
                            bias=bias2[:, p:p + 1], scale=1.0)
                    dst = out_d[2 * p:2 * p + 2].flatten_outer_dims()
                    lo, hi = w0 * CHUNK, (chunks[-1] + 1) * CHUNK
                    nc.sync.dma_start(out=dst[:, lo:hi], in_=outt[:, lo:hi])

    nc.compile()
    return nc


@functools.lru_cache(maxsize=1)
def _nc_cached():
    return build_nc()


def _prep_in_maps(inputs):
    x = np.asarray(inputs["x"], dtype=np.float32).reshape(B, CIN, PIX)
    rv = np.asarray(inputs["routing_vector"], dtype=np.float32)
    conv_w = np.asarray(inputs["conv_w"], dtype=np.float32)
    conv_b = np.asarray(inputs["conv_b"], dtype=np.float32)
    emb = np.asarray(inputs["emb"], dtype=np.float32)
    rp_w = np.asarray(inputs["rp_w"], dtype=np.float32)
    rp_b = np.asarray(inputs["rp_b"], dtype=np.float32)

    # base[cin, n, t*64+cout] = conv_w[n, cout, cin, dy, dx], t = dy*3+dx
    base = np.ascontiguousarray(
        conv_w.transpose(2, 0, 3, 4, 1).reshape(CIN, NEXP, TAPS * COUT))

    blob = np.zeros((128, CBLOB), np.float32)
    # rpw: blob[p, k*128+d] = rp_w[d, k*128+p]
    blob[:, C_RPW:C_RPW + 512] = (
        rp_w.T.reshape(4, 128, D).transpose(1, 0, 2).reshape(128, 512))
    blob[:, C_RPB] = rp_b
    blob[:, C_ID:C_ID + 128] = np.eye(128, dtype=np.float32)
    blob[0:NEXP, C_EMB:C_EMB + 128] = emb
    sel = np.zeros((B_LOC, NPAIR, 128), np.float32)
    for p in range(NPAIR):
        sel[2 * p, p, 0:64] = 1.0
        sel[2 * p + 1, p, 64:128] = 1.0
    blob[0:B_LOC, C_SEL:C_SEL + 256] = sel.reshape(B_LOC, 256)
    blob[:, C_CBT:C_CBT + NEXP] = np.tile(conv_b.T, (2, 1))

    in_maps = []
    for c in range(N_CORES):
        sl = slice(B_LOC * c, B_LOC * (c + 1))
        cblob = blob.copy()
        # rv: cblob[p, C_RV + k*4 + b] = rv[4c + b, k*128 + p]
        cblob[:, C_RV:C_RV + 16] = (
            rv[sl].T.reshape(4, 128, B_LOC).transpose(1, 0, 2).reshape(128, 16))
        in_maps.append({
            "x": np.concatenate([x[sl].reshape(-1),
                                 np.zeros(XPAD, np.float32)]),
            "cst": cblob,
            "base": base,
        })
    return in_maps


def run(inputs, trace=False, **kw):
    """Returns (full_output, BassKernelResults)."""
    nc = _nc_cached()
    in_maps = _prep_in_maps(inputs)
    res = run_bass_kernel_spmd(nc, in_maps, core_ids=list(range(N_CORES)),
                               trace=trace, **kw)
    outs = [r["out"].reshape(B_LOC, COUT, 64, 64)[:, :, :62, :62]
            for r in res.results]
    return np.concatenate(outs, axis=0), res


def kernel(**inputs):
    out, _ = run(inputs, trace=False)
    return out
